# revision 22
# baseline (speedup 1.0000x reference)
"""ELPH edge-aware GNN message passing on 8 Trainium2 NeuronCores.

Strategy (edge-parallel per the sharding hint, with a dst-sort refinement):
  - Sort edges by destination, shard by dst range: core c owns nodes
    [c*12500, (c+1)*12500) and all edges into them -> no cross-core reduce.
  - Edge-feature fold ("G-fold"): solve W1ab^T G = [W1c; b1]^T on the host
    (W1ab = the [128,128] endpoint block of W1) and stream
    s[e] = [x_src; x_dst] + G @ [log1p(ef); 1] in bf16. Then
    h = relu(W1ab^T s) exactly, so the message MLP's first layer is ONE
    K=128 matmul per 128-edge chunk (no separate edge-feature matmul).
  - Aggregation: edges grouped into 128-node dst windows; per chunk one
    matmul h^T @ A accumulates into the window's PSUM tile, with A the
    0/1 dst-selection matrix streamed from host as fp8 (exact).
  - PE scheduling: m1 of chunk c+16 interleaves 1:1 with agg of chunk c so
    consecutive matmuls never hit the same PSUM region (same-region
    accumulation serializes at ~213ns; interleaved sustains ~61ns/matmul).
  - relu: one wide [128,512] op per 4 chunks, alternating DVE/Act engines.
  - Update MLP: 4-block groups of 512 nodes with wide N=512 matmuls.
  - DMA: s-stream on the sync queue, A on the scalar queue, node/update
    traffic on the gpsimd queue.
"""
import numpy as np
import ml_dtypes

import concourse.bass as bass
import concourse.mybir as mybir
import concourse.tile as tile
from concourse import bacc
from concourse.bass_utils import run_bass_kernel_spmd

N_NODES = 100000
D_NODE = 64
D_EDGE = 4
H_MSG = 128
H_UPD = 128
N_CORES = 8
N_CORE = N_NODES // N_CORES           # 12500
BLK = 128
N_BLOCKS = (N_CORE + BLK - 1) // BLK  # 98
N_CORE_PAD = N_BLOCKS * BLK           # 12544
P = 128
ST = 16                               # chunks per supertile (2048 edges)
GRP = 4                               # blocks per update group (512 nodes)
N_GRP = (N_BLOCKS + GRP - 1) // GRP   # 25 (last group ragged: 2 blocks)

BF16 = mybir.dt.bfloat16
F32 = mybir.dt.float32
FP8 = mybir.dt.float8e4
nbf16 = ml_dtypes.bfloat16
nfp8 = ml_dtypes.float8_e4m3


def _install_trace_hook_if_possible():
    """Best-effort antenv.axon_hooks shim; only matters if BASS_TRACE is set."""
    import sys
    import types
    try:
        import antenv
        import antenv.axon_hooks  # noqa: F401
        return
    except Exception:
        pass
    try:
        import antenv
        from trn_agent_boot.trn_boot import _ntff_profile_via_ctypes
        mod = types.ModuleType("antenv.axon_hooks")
        mod._hook = _ntff_profile_via_ctypes("/opt/axon/libaxon_pjrt.so")
        mod.set_axon_ntff_profile_hook = lambda h: setattr(mod, "_hook", h)
        mod.get_axon_ntff_profile_hook = lambda: mod._hook
        sys.modules["antenv.axon_hooks"] = mod
        antenv.axon_hooks = mod
    except Exception:
        import os
        os.environ["BASS_NEVER_TRACE"] = "1"


def _build_program(chunk_meta, C, E_pad, with_deg, with_bu2):
    """chunk_meta[ci] = (slot, first, last). C % 16 == 0."""
    nc = bacc.Bacc("TRN2", target_bir_lowering=False, debug=False)

    sst = nc.declare_dram_parameter("sst", [P, E_pad], BF16, isOutput=False)
    amat = nc.declare_dram_parameter("amat", [P, E_pad], FP8, isOutput=False)
    xt = nc.declare_dram_parameter("xt", [D_NODE, N_CORE_PAD], BF16, isOutput=False)
    w1ab = nc.declare_dram_parameter("w1ab", [P, H_MSG], BF16, isOutput=False)
    u1a = nc.declare_dram_parameter("u1a", [D_NODE, H_UPD], BF16, isOutput=False)
    wc = nc.declare_dram_parameter("wc", [H_MSG, H_UPD], BF16, isOutput=False)
    u2 = nc.declare_dram_parameter("u2", [H_UPD, D_NODE], BF16, isOutput=False)
    bu1c = nc.declare_dram_parameter("bu1c", [H_UPD, 1], F32, isOutput=False)
    bu2c = nc.declare_dram_parameter("bu2c", [D_NODE, 1], F32, isOutput=False)
    if with_deg:
        vbrow = nc.declare_dram_parameter("vbrow", [1, H_UPD], BF16, isOutput=False)
        deg = nc.declare_dram_parameter("deg", [1, N_CORE_PAD], BF16, isOutput=False)
    outt = nc.declare_dram_parameter("outt", [D_NODE, N_CORE_PAD], F32, isOutput=True)

    n_st = (C + ST - 1) // ST
    LAG = 16

    with tile.TileContext(nc) as tc:
        with (
            tc.tile_pool(name="const", bufs=1) as cpool,
            tc.tile_pool(name="sstp", bufs=4) as sst_pool,
            tc.tile_pool(name="ap", bufs=4) as a_pool,
            tc.tile_pool(name="hq", bufs=12) as h_pool,
            tc.tile_pool(name="upd", bufs=2) as upd_pool,
            tc.tile_pool(name="peh", bufs=1, space="PSUM") as peh_pool,
            tc.tile_pool(name="pt1", bufs=1, space="PSUM") as pt1_pool,
            tc.tile_pool(name="pup", bufs=2, space="PSUM") as pup_pool,
        ):
            def cload(shape, dt_, param):
                t = cpool.tile(shape, dt_, tag=param.name, name=param.name + "_sb")
                nc.gpsimd.dma_start(out=t[:], in_=param[:])
                return t

            w1ab_sb = cpool.tile([P, H_MSG], BF16, tag="w1ab", name="w1ab_sb")

            # 4 peh banks, each holds 4 chunks [e, H]; 2 pt1 group banks.
            peh = [peh_pool.tile([P, 4, H_MSG], F32, space="PSUM", tag=f"peh{b}",
                                 name=f"peh{b}") for b in range(4)]
            pt1 = [pt1_pool.tile([H_MSG, GRP, BLK], F32, space="PSUM",
                                 tag=f"pt1{b}", name=f"pt1{b}") for b in range(2)]

            # slot -> (group, lane); slots are processed in ascending order.
            n_slots = max(s for s, _, _ in chunk_meta) + 1

            # stream tiles per supertile
            sst_tiles = {}
            a_tiles = {}

            def fetch_st(si):
                if si >= n_st:
                    return
                t = sst_pool.tile([P, ST * P], BF16, tag="sst", name=f"sst{si % 4}")
                nc.sync.dma_start(out=t[:], in_=sst[:, si * ST * P:(si + 1) * ST * P])
                a = a_pool.tile([P, ST * P], FP8, tag="amat", name=f"am{si % 4}")
                nc.sync.dma_start(out=a[:], in_=amat[:, si * ST * P:(si + 1) * ST * P])
                sst_tiles[si] = t
                a_tiles[si] = a

            fetch_st(0)
            nc.gpsimd.dma_start(out=w1ab_sb[:], in_=w1ab[:])
            fetch_st(1)
            fetch_st(2)
            u1a_sb = cload([D_NODE, H_UPD], BF16, u1a)
            wc_sb = cload([H_MSG, H_UPD], BF16, wc)
            u2_sb = cload([H_UPD, D_NODE], BF16, u2)
            bu1_sb = cload([H_UPD, 1], F32, bu1c)
            bu2_sb = cload([D_NODE, 1], F32, bu2c)
            if with_deg:
                vb_sb = cload([1, H_UPD], BF16, vbrow)
                deg_sb = cload([1, N_CORE_PAD], BF16, deg)

            h_tiles = {}   # bankgroup index (ci//4) -> h quad tile

            relu_pat = [0,1,0,1,0,1,0,1,0,1,1,0,1,0,1,0,1,0,1,1]
            relu_flip = [0]

            def emit_m1(ci):
                si, j = divmod(ci, ST)
                if j == 0:
                    fetch_st(si + 3)
                bank, sl = j // 4, j % 4
                nc.tensor.matmul(out=peh[bank][:, sl, :],
                                 lhsT=sst_tiles[si][:, j * P:(j + 1) * P],
                                 rhs=w1ab_sb[:], start=True, stop=True)
                if sl == 3:
                    hq = h_pool.tile([P, 4, H_MSG], FP8, tag="hq",
                                     name=f"hq{(ci // 4) % 12}")
                    src = peh[bank][:].rearrange("p a b -> p (a b)")
                    dst = hq[:].rearrange("p a b -> p (a b)")
                    if relu_pat[relu_flip[0] % 20] == 0:
                        nc.vector.tensor_scalar(out=dst, in0=src, scalar1=0.0,
                                                scalar2=None,
                                                op0=mybir.AluOpType.max)
                    else:
                        nc.scalar.activation(
                            out=dst, in_=src,
                            func=mybir.ActivationFunctionType.Relu)
                    relu_flip[0] += 1
                    h_tiles[ci // 4] = hq

            grp_done = []

            def post_agg(ci):
                slot, first, last = chunk_meta[ci]
                g, lane = divmod(slot, GRP)
                if last and (lane == GRP - 1 or slot == n_slots - 1):
                    grp_done.append(g)

            def emit_agg(ci):
                si, j = divmod(ci, ST)
                slot, first, last = chunk_meta[ci]
                g, lane = divmod(slot, GRP)
                hq = h_tiles[ci // 4]
                nc.tensor.matmul(out=pt1[g % 2][:, lane, :],
                                 lhsT=hq[:, ci % 4, :],
                                 rhs=a_tiles[si][:, j * P:(j + 1) * P],
                                 start=first, stop=last)
                post_agg(ci)

            def emit_agg_pair(ci):
                # ci even: aggregate chunks ci, ci+1 (same quad by parity)
                if ci + 1 >= len(chunk_meta):
                    emit_agg(ci)
                    return
                s0, f0, l0 = chunk_meta[ci]
                s1_, f1, l1 = chunk_meta[ci + 1]
                if s0 != s1_:
                    emit_agg(ci)
                    emit_agg(ci + 1)
                    return
                si, j = divmod(ci, ST)
                g, lane = divmod(s0, GRP)
                hq = h_tiles[ci // 4]
                nc.tensor.matmul(
                    out=pt1[g % 2][:, lane, :],
                    lhsT=hq[:, (ci % 4):(ci % 4) + 2, :],
                    rhs=a_tiles[si][:, j * P:(j + 2) * P].rearrange(
                        "p (a b) -> p a b", a=2),
                    start=f0, stop=l1,
                    perf_mode=mybir.MatmulPerfMode.DoubleRow)
                post_agg(ci)
                post_agg(ci + 1)

            # ---- update-phase stage machinery (per 4-block group) ----
            from collections import deque
            tailq = deque()

            def make_stages(g):
                gsz = min(GRP, n_slots - g * GRP)
                w = gsz * BLK
                n0 = g * GRP * BLK
                pt = pt1[g % 2]

                def s1():
                    xg = upd_pool.tile([D_NODE, GRP * BLK], BF16, tag="xg",
                                       name="xg", bufs=3)
                    nc.gpsimd.dma_start(out=xg[:, 0:w], in_=xt[:, n0:n0 + w])
                    t14 = upd_pool.tile([H_MSG, GRP * BLK], BF16, tag="t14",
                                        name="t14")
                    nc.scalar.activation(
                        out=t14[:, 0:w],
                        in_=pt[:].rearrange("p a b -> p (a b)")[:, 0:w],
                        func=mybir.ActivationFunctionType.Copy)
                    return (t14, xg)

                def s2(carry):
                    t14, xg_ = carry
                    puh = pup_pool.tile([H_UPD, GRP * BLK], F32, space="PSUM",
                                        tag="puh", name="puh", bufs=1)
                    nc.tensor.matmul(out=puh[:, 0:w], lhsT=u1a_sb[:],
                                     rhs=xg_[:, 0:w], start=True, stop=False)
                    return (t14, puh)

                def s3(carry):
                    t14, puh = carry
                    nc.tensor.matmul(out=puh[:, 0:w], lhsT=wc_sb[:],
                                     rhs=t14[:, 0:w], start=False,
                                     stop=not with_deg)
                    if with_deg:
                        nc.tensor.matmul(out=puh[:, 0:w], lhsT=vb_sb[:],
                                         rhs=deg_sb[:, n0:n0 + w],
                                         start=False, stop=True)
                    return (puh,)

                def s4(carry):
                    (puh,) = carry
                    ru = upd_pool.tile([H_UPD, GRP * BLK], BF16, tag="ru",
                                       name="ru")
                    nc.vector.tensor_scalar(out=ru[:, 0:w], in0=puh[:, 0:w],
                                            scalar1=bu1_sb[:, 0:1], scalar2=0.0,
                                            op0=mybir.AluOpType.add,
                                            op1=mybir.AluOpType.max)
                    return (ru,)

                def s5(carry):
                    (ru,) = carry
                    po = pup_pool.tile([D_NODE, GRP * BLK], F32, space="PSUM",
                                       tag="po", name="po", bufs=1)
                    nc.tensor.matmul(out=po[:, 0:w], lhsT=u2_sb[:],
                                     rhs=ru[:, 0:w], start=True, stop=True)
                    return (po,)

                def s6(carry):
                    (po,) = carry
                    osb = upd_pool.tile([D_NODE, GRP * BLK], F32, tag="osb",
                                        name="osb", bufs=3)
                    nc.vector.tensor_scalar(out=osb[:, 0:w], in0=po[:, 0:w],
                                            scalar1=bu2_sb[:, 0:1],
                                            scalar2=None,
                                            op0=mybir.AluOpType.add)
                    nc.scalar.dma_start(out=outt[:, n0:n0 + w],
                                        in_=osb[:, 0:w])
                    return None

                return deque([s1, s2, s3, s4, s5, s6])

            state = {"carry": None}

            def pump_tail():
                if not tailq:
                    return
                stages = tailq[0]
                fn = stages.popleft()
                if fn is not None:
                    state["carry"] = fn(state["carry"]) \
                        if state["carry"] is not None else fn()
                if not stages:
                    tailq.popleft()
                    state["carry"] = None

            # ---- main schedule: m1(ci) | agg(ci-LAG) 1:1 interleave ----
            for ci in range(C):
                emit_m1(ci)
                cj = ci - LAG
                if cj >= 0 and cj % 2 == 0:
                    emit_agg_pair(cj)
                while grp_done:
                    tailq.append(make_stages(grp_done.pop(0)))
                if ci % 3 == 0:
                    pump_tail()
            for cj in range(C - LAG, C):
                if cj % 2 == 0:
                    emit_agg_pair(cj)
                while grp_done:
                    tailq.append(make_stages(grp_done.pop(0)))
                pump_tail()
            while tailq or state["carry"] is not None:
                pump_tail()

    if not nc.is_finalized():
        nc.finalize()
    return nc


def kernel(x, edge_index, edge_features, W1, b1, W2, b2, U1, bu1, U2, bu2):
    x = np.asarray(x, dtype=np.float32)
    ei = np.asarray(edge_index).astype(np.int64)
    ef = np.asarray(edge_features, dtype=np.float32)
    W1 = np.asarray(W1, dtype=np.float64)
    b1 = np.asarray(b1, dtype=np.float64)
    src, dst = ei[0], ei[1]

    order = np.argsort(dst, kind="stable")
    src_s, dst_s, ef_s = src[order], dst[order], ef[order]

    core_of = dst_s // N_CORE
    blk_of = (dst_s % N_CORE) // BLK

    # per-(core, block) edge counts -> shared chunk schedule. Each core maps
    # its rank-k smallest block to slot k so the shared per-slot chunk count
    # is the max over aligned sorted profiles (small padding).
    cnt = np.zeros((N_CORES, N_BLOCKS), dtype=np.int64)
    np.add.at(cnt, (core_of, blk_of), 1)
    nbc = np.maximum(1, (cnt + P - 1) // P)
    blk_order = np.argsort(nbc, axis=1, kind="stable")   # ascending: slot->block
    sorted_nb = np.take_along_axis(nbc, blk_order, axis=1)
    NB = sorted_nb.max(axis=0)
    NB[-1] += (-NB.sum()) % 4
    C = int(NB.sum())
    E_pad = ((C + ST - 1) // ST) * ST * P
    blk_chunk0 = np.concatenate([[0], np.cumsum(NB)[:-1]])

    chunk_meta = []
    for s in range(N_BLOCKS):
        for j in range(int(NB[s])):
            chunk_meta.append((s, j == 0, j == int(NB[s]) - 1))

    # G-fold: W1ab^T G = [W1c ; b1]^T  (fp64 solve on host)
    W1ab = W1[:2 * D_NODE]
    rhs = np.concatenate([W1[2 * D_NODE:], b1.reshape(1, H_MSG)], axis=0).T
    G = np.linalg.solve(W1ab.T, rhs)             # [128, 5]
    assert np.all(np.isfinite(G))
    efp = np.log1p(ef_s, dtype=np.float32)       # [E, 4]
    delta = efp @ G[:, :D_EDGE].T.astype(np.float32)
    delta += G[:, D_EDGE].astype(np.float32)     # bias column  [E, 128]

    with_deg = bool(np.any(np.asarray(b2)))
    w1ab_h = np.ascontiguousarray(W1ab).astype(nbf16)
    U1 = np.asarray(U1, dtype=np.float64)
    u1a_h = np.ascontiguousarray(U1[:D_NODE]).astype(nbf16)
    wc_h = (np.asarray(W2, dtype=np.float64) @ U1[D_NODE:]).astype(nbf16)
    vb_h = (np.asarray(b2, dtype=np.float64) @ U1[D_NODE:]).reshape(
        1, H_UPD).astype(nbf16)
    u2_h = np.asarray(U2).astype(nbf16)
    bu1_h = np.asarray(bu1, dtype=np.float32).reshape(H_UPD, 1)
    bu2_h = np.asarray(bu2, dtype=np.float32).reshape(D_NODE, 1)

    in_maps = []
    for c in range(N_CORES):
        m = core_of == c
        eb = blk_of[m]
        first_pos = np.searchsorted(eb, np.arange(N_BLOCKS), side="left")
        rank = np.arange(eb.shape[0]) - first_pos[eb]
        slot_of_blk = np.empty(N_BLOCKS, dtype=np.int64)
        slot_of_blk[blk_order[c]] = np.arange(N_BLOCKS)
        slot = (blk_chunk0[slot_of_blk[eb]] * P + rank).astype(np.int64)

        e_src = src_s[m]
        e_dst = dst_s[m]
        e_delta = delta[m]

        sst_h = np.zeros((E_pad, 2 * D_NODE), dtype=np.float32)
        sst_h[slot, :D_NODE] = x[e_src]
        sst_h[slot, D_NODE:] = x[e_dst]
        sst_h[slot] += e_delta
        sst_h = np.ascontiguousarray(sst_h.T.astype(nbf16))

        amat_h = np.zeros((P, E_pad), dtype=nfp8)
        dstl = ((e_dst % N_CORE) % BLK).astype(np.int64)
        amat_h[slot % P, (slot // P) * P + dstl] = 1.0

        xt_h = np.zeros((N_CORE_PAD, D_NODE), dtype=nbf16)
        deg_h = np.zeros((1, N_CORE_PAD), dtype=nbf16)
        deg_n = np.bincount(e_dst % N_CORE, minlength=N_CORE_PAD).astype(np.float32)
        for s in range(N_BLOCKS):
            b = blk_order[c][s]
            n0, n1 = b * BLK, min(b * BLK + BLK, N_CORE)
            xt_h[s * BLK:s * BLK + (n1 - n0)] = \
                x[c * N_CORE + n0:c * N_CORE + n1].astype(nbf16)
            deg_h[0, s * BLK:s * BLK + (n1 - n0)] = deg_n[n0:n1]
        xt_h = np.ascontiguousarray(xt_h.T)

        im = {
            "sst": sst_h, "amat": amat_h, "xt": xt_h,
            "w1ab": w1ab_h, "u1a": u1a_h, "wc": wc_h, "u2": u2_h,
            "bu1c": bu1_h, "bu2c": bu2_h,
        }
        if with_deg:
            im["vbrow"] = vb_h
            im["deg"] = deg_h
        in_maps.append(im)

    _install_trace_hook_if_possible()
    with_bu2 = bool(np.any(np.asarray(bu2)))
    nc = _build_program(chunk_meta, C, E_pad, with_deg, with_bu2)
    res = run_bass_kernel_spmd(nc, in_maps, list(range(N_CORES)))
    global _last_results
    _last_results = res

    out = np.empty((N_NODES, D_NODE), dtype=np.float32)
    for c in range(N_CORES):
        ot = res.results[c]["outt"].T
        for s in range(N_BLOCKS):
            b = blk_order[c][s]
            n0, n1 = b * BLK, min(b * BLK + BLK, N_CORE)
            out[c * N_CORE + n0:c * N_CORE + n1] = ot[s * BLK:s * BLK + (n1 - n0)]
    return out


# revision 23
# speedup vs baseline: 1.0293x; 1.0293x over previous
"""ELPH edge-aware GNN message passing on 8 Trainium2 NeuronCores.

Strategy (edge-parallel per the sharding hint, with a dst-sort refinement):
  - Sort edges by destination, shard by dst range: core c owns nodes
    [c*12500, (c+1)*12500) and all edges into them -> no cross-core reduce.
  - Edge-feature fold ("G-fold"): solve W1ab^T G = [W1c; b1]^T on the host
    (W1ab = the [128,128] endpoint block of W1) and stream
    s[e] = [x_src; x_dst] + G @ [log1p(ef); 1] in bf16. Then
    h = relu(W1ab^T s) exactly, so the message MLP's first layer is ONE
    K=128 matmul per 128-edge chunk (no separate edge-feature matmul).
  - Aggregation: edges grouped into 128-node dst windows; per chunk one
    matmul h^T @ A accumulates into the window's PSUM tile, with A the
    0/1 dst-selection matrix streamed from host as fp8 (exact).
  - PE scheduling: m1 of chunk c+16 interleaves 1:1 with agg of chunk c so
    consecutive matmuls never hit the same PSUM region (same-region
    accumulation serializes at ~213ns; interleaved sustains ~61ns/matmul).
  - relu: one wide [128,512] op per 4 chunks, alternating DVE/Act engines.
  - Update MLP: 4-block groups of 512 nodes with wide N=512 matmuls.
  - DMA: s-stream on the sync queue, A on the scalar queue, node/update
    traffic on the gpsimd queue.
"""
import numpy as np
import ml_dtypes

import concourse.bass as bass
import concourse.mybir as mybir
import concourse.tile as tile
from concourse import bacc
from concourse.bass_utils import run_bass_kernel_spmd

N_NODES = 100000
D_NODE = 64
D_EDGE = 4
H_MSG = 128
H_UPD = 128
N_CORES = 8
N_CORE = N_NODES // N_CORES           # 12500
BLK = 128
N_BLOCKS = (N_CORE + BLK - 1) // BLK  # 98
N_CORE_PAD = N_BLOCKS * BLK           # 12544
P = 128
ST = 16                               # chunks per supertile (2048 edges)
GRP = 4                               # blocks per update group (512 nodes)
N_GRP = (N_BLOCKS + GRP - 1) // GRP   # 25 (last group ragged: 2 blocks)

BF16 = mybir.dt.bfloat16
F32 = mybir.dt.float32
FP8 = mybir.dt.float8e4
nbf16 = ml_dtypes.bfloat16
nfp8 = ml_dtypes.float8_e4m3


def _install_trace_hook_if_possible():
    """Best-effort antenv.axon_hooks shim; only matters if BASS_TRACE is set."""
    import sys
    import types
    try:
        import antenv
        import antenv.axon_hooks  # noqa: F401
        return
    except Exception:
        pass
    try:
        import antenv
        from trn_agent_boot.trn_boot import _ntff_profile_via_ctypes
        mod = types.ModuleType("antenv.axon_hooks")
        mod._hook = _ntff_profile_via_ctypes("/opt/axon/libaxon_pjrt.so")
        mod.set_axon_ntff_profile_hook = lambda h: setattr(mod, "_hook", h)
        mod.get_axon_ntff_profile_hook = lambda: mod._hook
        sys.modules["antenv.axon_hooks"] = mod
        antenv.axon_hooks = mod
    except Exception:
        import os
        os.environ["BASS_NEVER_TRACE"] = "1"


def _build_program(chunk_meta, C, E_pad, with_deg, with_bu2):
    """chunk_meta[ci] = (slot, first, last). C % 16 == 0."""
    nc = bacc.Bacc("TRN2", target_bir_lowering=False, debug=False)

    sst = nc.declare_dram_parameter("sst", [P, E_pad], BF16, isOutput=False)
    amat = nc.declare_dram_parameter("amat", [P, E_pad], FP8, isOutput=False)
    xt = nc.declare_dram_parameter("xt", [D_NODE, N_CORE_PAD], BF16, isOutput=False)
    w1ab = nc.declare_dram_parameter("w1ab", [P, H_MSG], BF16, isOutput=False)
    u1a = nc.declare_dram_parameter("u1a", [D_NODE, H_UPD], BF16, isOutput=False)
    wc = nc.declare_dram_parameter("wc", [H_MSG, H_UPD], BF16, isOutput=False)
    u2 = nc.declare_dram_parameter("u2", [H_UPD, D_NODE], BF16, isOutput=False)
    bu1c = nc.declare_dram_parameter("bu1c", [H_UPD, 1], F32, isOutput=False)
    bu2c = nc.declare_dram_parameter("bu2c", [D_NODE, 1], F32, isOutput=False)
    if with_deg:
        vbrow = nc.declare_dram_parameter("vbrow", [1, H_UPD], BF16, isOutput=False)
        deg = nc.declare_dram_parameter("deg", [1, N_CORE_PAD], BF16, isOutput=False)
    outt = nc.declare_dram_parameter("outt", [D_NODE, N_CORE_PAD], F32, isOutput=True)

    n_st = (C + ST - 1) // ST
    LAG = 16

    with tile.TileContext(nc) as tc:
        with (
            tc.tile_pool(name="const", bufs=1) as cpool,
            tc.tile_pool(name="sstp", bufs=4) as sst_pool,
            tc.tile_pool(name="ap", bufs=4) as a_pool,
            tc.tile_pool(name="hq", bufs=12) as h_pool,
            tc.tile_pool(name="upd", bufs=2) as upd_pool,
            tc.tile_pool(name="peh", bufs=1, space="PSUM") as peh_pool,
            tc.tile_pool(name="pt1", bufs=1, space="PSUM") as pt1_pool,
            tc.tile_pool(name="pup", bufs=2, space="PSUM") as pup_pool,
        ):
            def cload(shape, dt_, param):
                t = cpool.tile(shape, dt_, tag=param.name, name=param.name + "_sb")
                nc.gpsimd.dma_start(out=t[:], in_=param[:])
                return t

            w1ab_sb = cpool.tile([P, H_MSG], BF16, tag="w1ab", name="w1ab_sb")

            # 4 peh banks, each holds 4 chunks [e, H]; 2 pt1 group banks.
            peh = [peh_pool.tile([P, 4, H_MSG], F32, space="PSUM", tag=f"peh{b}",
                                 name=f"peh{b}") for b in range(4)]
            pt1 = [pt1_pool.tile([H_MSG, GRP, BLK], F32, space="PSUM",
                                 tag=f"pt1{b}", name=f"pt1{b}") for b in range(2)]

            # slot -> (group, lane); slots are processed in ascending order.
            n_slots = max(s for s, _, _ in chunk_meta) + 1

            # stream tiles per supertile
            sst_tiles = {}
            a_tiles = {}

            def fetch_st(si):
                if si >= n_st:
                    return
                t = sst_pool.tile([P, ST * P], BF16, tag="sst", name=f"sst{si % 4}")
                nc.sync.dma_start(out=t[:], in_=sst[:, si * ST * P:(si + 1) * ST * P])
                a = a_pool.tile([P, ST * P], FP8, tag="amat", name=f"am{si % 4}")
                nc.sync.dma_start(out=a[:], in_=amat[:, si * ST * P:(si + 1) * ST * P])
                sst_tiles[si] = t
                a_tiles[si] = a

            fetch_st(0)
            nc.gpsimd.dma_start(out=w1ab_sb[:], in_=w1ab[:])
            fetch_st(1)
            fetch_st(2)
            u1a_sb = cload([D_NODE, H_UPD], BF16, u1a)
            wc_sb = cload([H_MSG, H_UPD], BF16, wc)
            u2_sb = cload([H_UPD, D_NODE], BF16, u2)
            bu1_sb = cload([H_UPD, 1], F32, bu1c)
            bu2_sb = cload([D_NODE, 1], F32, bu2c)
            if with_deg:
                vb_sb = cload([1, H_UPD], BF16, vbrow)
                deg_sb = cload([1, N_CORE_PAD], BF16, deg)

            h_tiles = {}   # bankgroup index (ci//4) -> h quad tile

            relu_pat = [0,1,0,1,0,1,0,1,0,1,1,0,1,0,1,0,1,0,1,1]
            relu_flip = [0]

            def emit_m1(ci):
                si, j = divmod(ci, ST)
                if j == 0:
                    fetch_st(si + 3)
                bank, sl = j // 4, j % 4
                nc.tensor.matmul(out=peh[bank][:, sl, :],
                                 lhsT=sst_tiles[si][:, j * P:(j + 1) * P],
                                 rhs=w1ab_sb[:], start=True, stop=True)
                if sl == 3:
                    hq = h_pool.tile([P, 4, H_MSG], FP8, tag="hq",
                                     name=f"hq{(ci // 4) % 12}")
                    src = peh[bank][:].rearrange("p a b -> p (a b)")
                    dst = hq[:].rearrange("p a b -> p (a b)")
                    if relu_pat[relu_flip[0] % 20] == 0:
                        nc.vector.tensor_scalar(out=dst, in0=src, scalar1=0.0,
                                                scalar2=None,
                                                op0=mybir.AluOpType.max)
                    else:
                        nc.scalar.activation(
                            out=dst, in_=src,
                            func=mybir.ActivationFunctionType.Relu)
                    relu_flip[0] += 1
                    h_tiles[ci // 4] = hq

            grp_done = []

            def post_agg(ci):
                slot, first, last = chunk_meta[ci]
                g, lane = divmod(slot, GRP)
                if last and (lane == GRP - 1 or slot == n_slots - 1):
                    grp_done.append(g)

            def emit_agg(ci):
                si, j = divmod(ci, ST)
                slot, first, last = chunk_meta[ci]
                g, lane = divmod(slot, GRP)
                hq = h_tiles[ci // 4]
                nc.tensor.matmul(out=pt1[g % 2][:, lane, :],
                                 lhsT=hq[:, ci % 4, :],
                                 rhs=a_tiles[si][:, j * P:(j + 1) * P],
                                 start=first, stop=last)
                post_agg(ci)

            def emit_agg_pair(ci):
                # ci even: aggregate chunks ci, ci+1 (same quad by parity)
                if ci + 1 >= len(chunk_meta):
                    emit_agg(ci)
                    return
                s0, f0, l0 = chunk_meta[ci]
                s1_, f1, l1 = chunk_meta[ci + 1]
                if s0 != s1_:
                    emit_agg(ci)
                    emit_agg(ci + 1)
                    return
                si, j = divmod(ci, ST)
                g, lane = divmod(s0, GRP)
                hq = h_tiles[ci // 4]
                nc.tensor.matmul(
                    out=pt1[g % 2][:, lane, :],
                    lhsT=hq[:, (ci % 4):(ci % 4) + 2, :],
                    rhs=a_tiles[si][:, j * P:(j + 2) * P].rearrange(
                        "p (a b) -> p a b", a=2),
                    start=f0, stop=l1,
                    perf_mode=mybir.MatmulPerfMode.DoubleRow)
                post_agg(ci)
                post_agg(ci + 1)

            # ---- update-phase stage machinery (per 4-block group) ----
            from collections import deque
            tailq = deque()

            def make_stages(g):
                gsz = min(GRP, n_slots - g * GRP)
                w = gsz * BLK
                n0 = g * GRP * BLK
                pt = pt1[g % 2]

                def s1():
                    xg = upd_pool.tile([D_NODE, GRP * BLK], BF16, tag="xg",
                                       name="xg", bufs=3)
                    nc.gpsimd.dma_start(out=xg[:, 0:w], in_=xt[:, n0:n0 + w])
                    t14 = upd_pool.tile([H_MSG, GRP * BLK], BF16, tag="t14",
                                        name="t14")
                    nc.scalar.activation(
                        out=t14[:, 0:w],
                        in_=pt[:].rearrange("p a b -> p (a b)")[:, 0:w],
                        func=mybir.ActivationFunctionType.Copy)
                    return (t14, xg)

                def s2(carry):
                    t14, xg_ = carry
                    puh = pup_pool.tile([H_UPD, GRP * BLK], F32, space="PSUM",
                                        tag="puh", name="puh", bufs=1)
                    nc.tensor.matmul(out=puh[:, 0:w], lhsT=u1a_sb[:],
                                     rhs=xg_[:, 0:w], start=True, stop=False)
                    return (t14, puh)

                def s3(carry):
                    t14, puh = carry
                    nc.tensor.matmul(out=puh[:, 0:w], lhsT=wc_sb[:],
                                     rhs=t14[:, 0:w], start=False,
                                     stop=not with_deg)
                    if with_deg:
                        nc.tensor.matmul(out=puh[:, 0:w], lhsT=vb_sb[:],
                                         rhs=deg_sb[:, n0:n0 + w],
                                         start=False, stop=True)
                    return (puh,)

                def s4(carry):
                    (puh,) = carry
                    ru = upd_pool.tile([H_UPD, GRP * BLK], BF16, tag="ru",
                                       name="ru")
                    nc.vector.tensor_scalar(out=ru[:, 0:w], in0=puh[:, 0:w],
                                            scalar1=bu1_sb[:, 0:1], scalar2=0.0,
                                            op0=mybir.AluOpType.add,
                                            op1=mybir.AluOpType.max)
                    return (ru,)

                def s5(carry):
                    (ru,) = carry
                    po = pup_pool.tile([D_NODE, GRP * BLK], F32, space="PSUM",
                                       tag="po", name="po", bufs=1)
                    nc.tensor.matmul(out=po[:, 0:w], lhsT=u2_sb[:],
                                     rhs=ru[:, 0:w], start=True, stop=True)
                    return (po,)

                def s6(carry):
                    (po,) = carry
                    osb = upd_pool.tile([D_NODE, GRP * BLK], F32, tag="osb",
                                        name="osb", bufs=3)
                    nc.vector.tensor_scalar(out=osb[:, 0:w], in0=po[:, 0:w],
                                            scalar1=bu2_sb[:, 0:1],
                                            scalar2=None,
                                            op0=mybir.AluOpType.add)
                    nc.gpsimd.dma_start(out=outt[:, n0:n0 + w],
                                        in_=osb[:, 0:w])
                    return None

                return deque([s1, s2, s3, s4, s5, s6])

            state = {"carry": None}

            def pump_tail():
                if not tailq:
                    return
                stages = tailq[0]
                fn = stages.popleft()
                if fn is not None:
                    state["carry"] = fn(state["carry"]) \
                        if state["carry"] is not None else fn()
                if not stages:
                    tailq.popleft()
                    state["carry"] = None

            # ---- main schedule: m1(ci) | agg(ci-LAG) 1:1 interleave ----
            for ci in range(C):
                emit_m1(ci)
                cj = ci - LAG
                if cj >= 0 and cj % 2 == 0:
                    emit_agg_pair(cj)
                while grp_done:
                    tailq.append(make_stages(grp_done.pop(0)))
                if ci % 3 == 0:
                    pump_tail()
            for cj in range(C - LAG, C):
                if cj % 2 == 0:
                    emit_agg_pair(cj)
                while grp_done:
                    tailq.append(make_stages(grp_done.pop(0)))
                pump_tail()
            while tailq or state["carry"] is not None:
                pump_tail()

    if not nc.is_finalized():
        nc.finalize()
    return nc


def kernel(x, edge_index, edge_features, W1, b1, W2, b2, U1, bu1, U2, bu2):
    x = np.asarray(x, dtype=np.float32)
    ei = np.asarray(edge_index).astype(np.int64)
    ef = np.asarray(edge_features, dtype=np.float32)
    W1 = np.asarray(W1, dtype=np.float64)
    b1 = np.asarray(b1, dtype=np.float64)
    src, dst = ei[0], ei[1]

    order = np.argsort(dst, kind="stable")
    src_s, dst_s, ef_s = src[order], dst[order], ef[order]

    core_of = dst_s // N_CORE
    blk_of = (dst_s % N_CORE) // BLK

    # per-(core, block) edge counts -> shared chunk schedule. Each core maps
    # its rank-k smallest block to slot k so the shared per-slot chunk count
    # is the max over aligned sorted profiles (small padding).
    cnt = np.zeros((N_CORES, N_BLOCKS), dtype=np.int64)
    np.add.at(cnt, (core_of, blk_of), 1)
    nbc = np.maximum(1, (cnt + P - 1) // P)
    blk_order = np.argsort(nbc, axis=1, kind="stable")   # ascending: slot->block
    sorted_nb = np.take_along_axis(nbc, blk_order, axis=1)
    NB = sorted_nb.max(axis=0)
    NB[-1] += (-NB.sum()) % 4
    C = int(NB.sum())
    E_pad = ((C + ST - 1) // ST) * ST * P
    blk_chunk0 = np.concatenate([[0], np.cumsum(NB)[:-1]])

    chunk_meta = []
    for s in range(N_BLOCKS):
        for j in range(int(NB[s])):
            chunk_meta.append((s, j == 0, j == int(NB[s]) - 1))

    # G-fold: W1ab^T G = [W1c ; b1]^T  (fp64 solve on host)
    W1ab = W1[:2 * D_NODE]
    rhs = np.concatenate([W1[2 * D_NODE:], b1.reshape(1, H_MSG)], axis=0).T
    G = np.linalg.solve(W1ab.T, rhs)             # [128, 5]
    assert np.all(np.isfinite(G))
    efp = np.log1p(ef_s, dtype=np.float32)       # [E, 4]
    delta = efp @ G[:, :D_EDGE].T.astype(np.float32)
    delta += G[:, D_EDGE].astype(np.float32)     # bias column  [E, 128]

    with_deg = bool(np.any(np.asarray(b2)))
    w1ab_h = np.ascontiguousarray(W1ab).astype(nbf16)
    U1 = np.asarray(U1, dtype=np.float64)
    u1a_h = np.ascontiguousarray(U1[:D_NODE]).astype(nbf16)
    wc_h = (np.asarray(W2, dtype=np.float64) @ U1[D_NODE:]).astype(nbf16)
    vb_h = (np.asarray(b2, dtype=np.float64) @ U1[D_NODE:]).reshape(
        1, H_UPD).astype(nbf16)
    u2_h = np.asarray(U2).astype(nbf16)
    bu1_h = np.asarray(bu1, dtype=np.float32).reshape(H_UPD, 1)
    bu2_h = np.asarray(bu2, dtype=np.float32).reshape(D_NODE, 1)

    in_maps = []
    for c in range(N_CORES):
        m = core_of == c
        eb = blk_of[m]
        first_pos = np.searchsorted(eb, np.arange(N_BLOCKS), side="left")
        rank = np.arange(eb.shape[0]) - first_pos[eb]
        slot_of_blk = np.empty(N_BLOCKS, dtype=np.int64)
        slot_of_blk[blk_order[c]] = np.arange(N_BLOCKS)
        slot = (blk_chunk0[slot_of_blk[eb]] * P + rank).astype(np.int64)

        e_src = src_s[m]
        e_dst = dst_s[m]
        e_delta = delta[m]

        sst_h = np.zeros((E_pad, 2 * D_NODE), dtype=np.float32)
        sst_h[slot, :D_NODE] = x[e_src]
        sst_h[slot, D_NODE:] = x[e_dst]
        sst_h[slot] += e_delta
        sst_h = np.ascontiguousarray(sst_h.T.astype(nbf16))

        amat_h = np.zeros((P, E_pad), dtype=nfp8)
        dstl = ((e_dst % N_CORE) % BLK).astype(np.int64)
        amat_h[slot % P, (slot // P) * P + dstl] = 1.0

        xt_h = np.zeros((N_CORE_PAD, D_NODE), dtype=nbf16)
        deg_h = np.zeros((1, N_CORE_PAD), dtype=nbf16)
        deg_n = np.bincount(e_dst % N_CORE, minlength=N_CORE_PAD).astype(np.float32)
        for s in range(N_BLOCKS):
            b = blk_order[c][s]
            n0, n1 = b * BLK, min(b * BLK + BLK, N_CORE)
            xt_h[s * BLK:s * BLK + (n1 - n0)] = \
                x[c * N_CORE + n0:c * N_CORE + n1].astype(nbf16)
            deg_h[0, s * BLK:s * BLK + (n1 - n0)] = deg_n[n0:n1]
        xt_h = np.ascontiguousarray(xt_h.T)

        im = {
            "sst": sst_h, "amat": amat_h, "xt": xt_h,
            "w1ab": w1ab_h, "u1a": u1a_h, "wc": wc_h, "u2": u2_h,
            "bu1c": bu1_h, "bu2c": bu2_h,
        }
        if with_deg:
            im["vbrow"] = vb_h
            im["deg"] = deg_h
        in_maps.append(im)

    _install_trace_hook_if_possible()
    with_bu2 = bool(np.any(np.asarray(bu2)))
    nc = _build_program(chunk_meta, C, E_pad, with_deg, with_bu2)
    res = run_bass_kernel_spmd(nc, in_maps, list(range(N_CORES)))
    global _last_results
    _last_results = res

    out = np.empty((N_NODES, D_NODE), dtype=np.float32)
    for c in range(N_CORES):
        ot = res.results[c]["outt"].T
        for s in range(N_BLOCKS):
            b = blk_order[c][s]
            n0, n1 = b * BLK, min(b * BLK + BLK, N_CORE)
            out[c * N_CORE + n0:c * N_CORE + n1] = ot[s * BLK:s * BLK + (n1 - n0)]
    return out
